# revision 1
# baseline (speedup 1.0000x reference)
"""Trainium2 Bass kernel for nn_MemorizingTransformer (retrieval_knn).

Memorizing-transformer attention block: cosine-sim causal local attention with
per-query retrieved KNN memories, joint softmax over [memory | local], and
input/output projections.

Sharding: (b, h) across 8 cores — core c handles batch b=c//4 and heads
h0=2*(c%4), h0+1. Every core runs an identical NEFF (pure SPMD); only input
slices differ. The output projection is computed per-core on the core's two
head rows of w_out, giving partial sums that the host reduces.

Device algorithm per core (f32, optional f32r matmuls):
  phase X : xT = transpose(x[b])                      (PE transposes)
  phase KV: k_nat, v = x@w_k, x@w_v; sumsq(k)         (PE, ACT)
  phase Q : q_nat = x@w_q_h (both heads); sumsq(q)    (PE, ACT)
  norm    : one batched sqrt + reciprocal; scale k,q; kT/qT via PE transpose
  per head p:
    mem scores per q-block g:
      S_mem = sum_d(mem_k * q_s)                      (DVE mul + seg reduce)
      P_mem, rowsum = exp(scale*S_mem - scale)        (ACT fused accum)
    local, jt-outer with 4-block column batching:
      S_T[128j, <=512q] = kT.T @ qT_all               (PE)
      P_T = exp(scale*S_T - scale), tril on diagonal  (ACT, DVE)
      PV: psum_o[g] += P_T_g.T @ [v|1]                (PE accum) [128q,4,65]
    mem values per g (PE, block-diagonal trick):
      mv_t[(ql j), g4, d] staged so 4 stride-32 queries stack on partitions;
      stage/stageT hold P_mem block-diagonally; 32 small matmuls give
      pm[65, 128q] = [mem_v|1].T @ P_mem per query; transposed-accumulated
      into psum_o so col 64 = total softmax denominator.
    combine: oh = psum_o[:, :64] * recip(psum_o[:, 64]); hoT = transpose(oh)
  partial_out[g] = hoT_g.T @ w_out[2 head rows]       (PE)

Softmax needs no max-subtraction: scores are cosine sims in [-1,1] times
scale=exp(scale_param), so exp(scale*(s-1)) is bounded in (0, 1].
"""

import os
import numpy as np

HEADS = 8
D = 64
KNN = 32
B = 2
N = 2048
DIM = 512
P = 128
NB = N // P          # 16 query/key blocks
NCO = DIM // P       # 4 contraction chunks of the model dim
NCORES = 8
FP32R = bool(int(os.environ.get("BASS_FP32R", "1")))
MEMBF16 = bool(int(os.environ.get("BASS_MEMBF16", "0")))
_SKIP_LOCAL = bool(int(os.environ.get("SKIP_LOCAL", "0")))
_SKIP_MEMK = bool(int(os.environ.get("SKIP_MEMK", "0")))
_SKIP_MEMV = bool(int(os.environ.get("SKIP_MEMV", "0")))
PHASE_MARKS = []
_MSTATE = {}


def _mark(nc, name):
    cur = nc.next_id()
    if _MSTATE.get("name") is not None:
        PHASE_MARKS.append((_MSTATE["name"], _MSTATE["id"], cur))
    _MSTATE["name"] = name
    _MSTATE["id"] = cur


def _build(use_mbias: bool):
    import concourse.bass as bass
    import concourse.mybir as mybir
    import concourse.tile as tile
    from concourse import bacc

    f32 = mybir.dt.float32
    f32r = mybir.dt.float32r
    bf16 = mybir.dt.bfloat16
    mdt = bf16 if MEMBF16 else f32
    AX = mybir.AxisListType
    ACTF = mybir.ActivationFunctionType

    def rcast(ap):
        return ap.bitcast(f32r) if FP32R else ap

    nc = bacc.Bacc(None, target_bir_lowering=False, name="memxformer")
    PHASE_MARKS.clear()
    _MSTATE.clear()

    # ---- I/O ------------------------------------------------------------
    xb = nc.dram_tensor("xb", (N, DIM), f32, kind="ExternalInput")
    wq2 = nc.dram_tensor("wq2", (DIM, 2 * D), f32, kind="ExternalInput")
    wkv = nc.dram_tensor("wkv", (DIM, 2 * D), f32, kind="ExternalInput")
    wout2 = nc.dram_tensor("wout2", (2 * D, DIM), f32, kind="ExternalInput")
    # scales[:, 0:2] = exp(scale_param[h0 + p]); scales[:, 2:4] = -that
    scales = nc.dram_tensor("scales", (P, 4), f32, kind="ExternalInput")
    mk = nc.dram_tensor("mk", (2, NB, P, KNN, D), f32, kind="ExternalInput")
    mv = nc.dram_tensor("mv", (2, NB, P, KNN, D + 1), mdt, kind="ExternalInput")
    if use_mbias:
        mbias = nc.dram_tensor("mbias", (2, NB, P, KNN), f32, kind="ExternalInput")
    out = nc.dram_tensor("out", (N, DIM), f32, kind="ExternalOutput")

    # constants baked into the NEFF
    eye_np = np.eye(P, dtype=np.float32)
    tril_np = np.triu(np.ones((P, P), dtype=np.float32))  # keep j <= q
    eye_d = nc.inline_tensor(eye_np, name="eye_c")
    import ml_dtypes
    eye16_d = nc.inline_tensor(eye_np.astype(ml_dtypes.bfloat16), name="eye16_c")
    tril_d = nc.inline_tensor(tril_np, name="tril_c")

    with tile.TileContext(nc) as tc:
        with (
            tc.tile_pool(name="singles", bufs=1) as singles,
            tc.tile_pool(name="xin", bufs=4) as xin,
            tc.tile_pool(name="mem", bufs=3) as memp,
            tc.tile_pool(name="mvp", bufs=3) as mvp,
            tc.tile_pool(name="prods", bufs=2) as prods,
            tc.tile_pool(name="small", bufs=6) as small,
            tc.tile_pool(name="pt", bufs=3) as ptp,
            tc.tile_pool(name="stts", bufs=3) as stts,
            tc.tile_pool(name="pms", bufs=3) as pms,
            tc.tile_pool(name="outp", bufs=3) as outp,
            tc.tile_pool(name="ppt", bufs=2, space="PSUM") as ppt,
            tc.tile_pool(name="pp512", bufs=2, space="PSUM") as pp512,
            tc.tile_pool(name="ppo", bufs=4, space="PSUM") as ppo,
        ):
            # ---- constants / weights ------------------------------------
            eye_sb = singles.tile([P, P], f32, tag="eye")
            nc.sync.dma_start(eye_sb, eye_d[:, :])
            eye16_sb = singles.tile([P, P], bf16, tag="eye16")
            nc.sync.dma_start(eye16_sb, eye16_d[:, :])
            tril_sb = singles.tile([P, P], f32, tag="tril")
            nc.sync.dma_start(tril_sb, tril_d[:, :])
            sc_sb = singles.tile([P, 4], f32, tag="scales")
            nc.sync.dma_start(sc_sb, scales[:, :])
            wq_sb = singles.tile([P, NCO, 2 * D], f32, tag="wq")
            nc.sync.dma_start(wq_sb, wq2[:, :].rearrange("(co p) c -> p co c", p=P))
            wkv_sb = singles.tile([P, NCO, 2 * D], f32, tag="wkv")
            nc.sync.dma_start(wkv_sb, wkv[:, :].rearrange("(co p) c -> p co c", p=P))
            wout_st = singles.tile([P, DIM], f32, tag="wout_st")
            nc.sync.dma_start(wout_st, wout2[:, :])
            wout_sb = singles.tile([P, DIM], f32r if FP32R else f32, tag="wout")
            nc.scalar.copy(out=wout_sb, in_=wout_st)

            _mark(nc, "setup")
            # ---- x transpose: xT[p, co, n] = x[n, co*128 + p] ------------
            xT = singles.tile([P, NCO, N], f32, tag="xT")
            for nb in range(NB):
                x_t = xin.tile([P, DIM], f32, tag="xtile")
                nc.sync.dma_start(x_t, xb[nb * P:(nb + 1) * P, :])
                pt_ps = ppt.tile([P, NCO, P], f32, tag="tps")
                for co in range(NCO):
                    nc.tensor.transpose(pt_ps[:, co, :],
                                        x_t[:, co * P:(co + 1) * P], eye_sb)
                nc.scalar.copy(out=xT[:, :, nb * P:(nb + 1) * P], in_=pt_ps)

            _mark(nc, "xT")
            # ---- k/v/q natural projections + sumsq ----------------------
            k_all = singles.tile([P, NB, D], f32, tag="k_all")
            v_aug = singles.tile([P, NB, D + 1], f32, tag="vaug")
            nc.gpsimd.memset(v_aug[:, :, D:D + 1], 1.0)
            q_all = singles.tile([P, 2 * NB, D], f32, tag="q_all")
            # ss_all: cols 0:16 = k blocks, 16:48 = q blocks (2 heads x 16)
            ss_all = singles.tile([P, NB + 2 * NB], f32, tag="ss")
            junk = singles.tile([P, D], f32, tag="junk")

            for g in range(NB):
                qsl = slice(g * P, (g + 1) * P)
                qnat = ppt.tile([P, 2 * D], f32, tag="tps")
                for co in range(NCO):
                    nc.tensor.matmul(qnat, xT[:, co, qsl], wq_sb[:, co, :],
                                     start=(co == 0), stop=(co == NCO - 1))
                for p in range(2):
                    idx = p * NB + g
                    nc.scalar.copy(out=q_all[:, idx, :],
                                   in_=qnat[:, p * D:(p + 1) * D])
                    nc.scalar.activation(out=junk, in_=qnat[:, p * D:(p + 1) * D],
                                         func=ACTF.Square,
                                         accum_out=ss_all[:, NB + idx:NB + idx + 1])

            _mark(nc, "kvqnat")
            # q norm scale first (memk can start as soon as q_s is ready)
            nrm_q = singles.tile([P, 2 * NB], f32, tag="nrm_q")
            nc.scalar.sqrt(nrm_q, ss_all[:, NB:3 * NB])
            rn_q = singles.tile([P, 2 * NB], f32, tag="rn_q")
            nc.vector.reciprocal(rn_q, nrm_q)
            q_s = singles.tile([P, 2 * NB, D], f32, tag="q_s")
            qT_all = singles.tile([D, 2 * NB, P], f32r if FP32R else f32, tag="qT")
            for idxc in range(0, 2 * NB, 4):
                qt_ps = ppt.tile([D, 4, P], f32, tag="tps")
                for i4 in range(4):
                    idx = idxc + i4
                    nc.vector.tensor_scalar_mul(q_s[:, idx, :], q_all[:, idx, :],
                                                rn_q[:, idx:idx + 1])
                    nc.tensor.transpose(qt_ps[:, i4, :], q_s[:, idx, :], eye_sb)
                nc.scalar.copy(out=qT_all[:, idxc:idxc + 4, :], in_=qt_ps)

            for jt in range(NB):
                ksl = slice(jt * P, (jt + 1) * P)
                kvnat = ppt.tile([P, 2 * D], f32, tag="tps")
                for co in range(NCO):
                    nc.tensor.matmul(kvnat, xT[:, co, ksl], wkv_sb[:, co, :],
                                     start=(co == 0), stop=(co == NCO - 1))
                nc.scalar.copy(out=k_all[:, jt, :], in_=kvnat[:, 0:D])
                nc.scalar.activation(out=junk, in_=kvnat[:, 0:D], func=ACTF.Square,
                                     accum_out=ss_all[:, jt:jt + 1])
                nc.scalar.copy(out=v_aug[:, jt, 0:D], in_=kvnat[:, D:2 * D])

            nrm_k = singles.tile([P, NB], f32, tag="nrm_k")
            nc.scalar.sqrt(nrm_k, ss_all[:, 0:NB])
            rn_k = singles.tile([P, NB], f32, tag="rn_k")
            nc.vector.reciprocal(rn_k, nrm_k)
            kT = singles.tile([D, NB, P], f32r if FP32R else f32, tag="kT")
            for jtc in range(0, NB, 4):
                kt_ps = ppt.tile([D, 4, P], f32, tag="tps")
                for j4 in range(4):
                    jt = jtc + j4
                    ktmp = small.tile([P, D], f32, tag="ktmp")
                    nc.vector.tensor_scalar_mul(ktmp, k_all[:, jt, :],
                                                rn_k[:, jt:jt + 1])
                    nc.tensor.transpose(kt_ps[:, j4, :], ktmp, eye_sb)
                nc.scalar.copy(out=kT[:, jtc:jtc + 4, :], in_=kt_ps)

            _mark(nc, "norm")
            # ---- head-output accumulator --------------------------------
            hoT = singles.tile([P, NB, P], f32r if FP32R else f32, tag="hoT")
            # staging for block-diagonal P_mem (manual double buffer; the
            # off-diagonal zeros are written once and never touched again)
            st2 = singles.tile([P, 2, 4, P], mdt, tag="st2")
            nc.gpsimd.memset(st2, 0.0)

            for p in range(2):
                sc_ap = sc_sb[:, p:p + 1]
                nb_ap = sc_sb[:, 2 + p:3 + p]

                _mark(nc, f"memk")
                # --- memory attention scores for all 16 blocks ---
                p_mem_all = singles.tile([P, NB, KNN], mdt, tag=f"pmem{p}")
                for g in range(NB if not _SKIP_MEMK else 0):
                    idx = p * NB + g
                    mk_t = memp.tile([P, KNN, D], f32, tag="mk")
                    nc.sync.dma_start(mk_t, mk[p, g])
                    prod = prods.tile([P, KNN, D], f32, tag="prod")
                    nc.vector.tensor_mul(
                        prod, mk_t, q_s[:, idx, None, :].to_broadcast((P, KNN, D)))
                    s_mem = small.tile([P, KNN], f32, tag="smem")
                    nc.vector.reduce_sum(s_mem, prod, axis=AX.X)
                    if use_mbias:
                        mb_t = small.tile([P, KNN], f32, tag="mbias")
                        nc.sync.dma_start(mb_t, mbias[p, g])
                        nc.vector.tensor_add(s_mem, s_mem, mb_t)
                    nc.scalar.activation(out=p_mem_all[:, g, :], in_=s_mem,
                                         func=ACTF.Exp, bias=nb_ap, scale=sc_ap)

                _mark(nc, f"local")
                # --- local causal attention, jt-outer, 4-block columns ---
                psum_o = [ppo.tile([P, 4, D + 1], f32, tag="po", name=f"po{i}")
                           for i in range(4)]
                def local_tile(qc, jt):
                    g_lo = max(jt, 4 * qc)
                    g_hi = 4 * qc + 4
                    ng = g_hi - g_lo
                    i_lo = p * NB + g_lo
                    st_ps = pp512.tile([P, 512], f32, tag="st", name="st_ps")
                    nc.tensor.matmul(
                        st_ps[:, :ng * P], kT[:, jt, :],
                        qT_all[:, i_lo:i_lo + ng, :],
                        start=True, stop=True)
                    p_t = ptp.tile([P, 4, P], f32, tag="pt", name="p_t")
                    nc.scalar.activation(
                        out=p_t[:, :ng, :],
                        in_=st_ps[:, :ng * P].rearrange("p (g q) -> p g q", q=P),
                        func=ACTF.Exp, bias=nb_ap, scale=sc_ap)
                    if g_lo <= jt < g_hi:
                        di = jt - g_lo
                        nc.vector.tensor_mul(p_t[:, di, :], p_t[:, di, :],
                                             tril_sb)
                    for gi in range(ng):
                        g = g_lo + gi
                        nc.tensor.matmul(
                            psum_o[qc][:, g - 4 * qc, :], p_t[:, gi, :],
                            v_aug[:, jt, :],
                            start=(jt == 0 and gi == 0), stop=False)

                _mark(nc, f"memv")
                # qc-outer: each bank's local attention completes, then its
                # memory-value chain fires immediately (overlaps later banks)
                for qc4 in range(4):
                    if not _SKIP_LOCAL:
                        for jt in range(4 * qc4 + 4):
                            local_tile(qc4, jt)
                    gc = 4 * qc4
                    if _SKIP_MEMK:
                        oh_ps0 = ppt.tile([D, 4, P], f32, tag="tps")
                        for gi in range(4):
                            g = gc + gi
                            qc, gq = g // 4, g % 4
                            rcp = small.tile([P, 1], f32, tag="rcp")
                            nc.vector.reciprocal(rcp, psum_o[qc][:, gq, D:D + 1])
                            oh = small.tile([P, D], f32, tag="oh")
                            nc.vector.tensor_scalar_mul(
                                oh, psum_o[qc][:, gq, 0:D], rcp)
                            nc.tensor.transpose(oh_ps0[:, gi, :], oh, eye_sb)
                        nc.scalar.copy(out=hoT[p * D:(p + 1) * D, gc:gc + 4, :],
                                       in_=oh_ps0)
                        continue
                    stage4 = st2[:, (gc // 4) % 2, :, :]
                    for gi in range(4):
                        g = gc + gi
                        for k4 in range(4):
                            nc.gpsimd.tensor_copy(
                                out=stage4[32 * k4:32 * (k4 + 1), gi,
                                           32 * k4:32 * (k4 + 1)],
                                in_=p_mem_all[32 * k4:32 * (k4 + 1), g, :])
                    stt_ps = ppt.tile([P, 4, P], mdt, tag="tps")
                    for gi in range(4):
                        nc.tensor.transpose(stt_ps[:, gi, :], stage4[:, gi, :],
                                            eye16_sb if MEMBF16 else eye_sb)
                    stT = stts.tile([P, 4, P], mdt, tag="stT")
                    nc.scalar.copy(out=stT, in_=stt_ps)
                    pm_ps = pp512.tile([D + 1, 4, P], f32, tag="st")
                    for gi in range(4):
                        g = gc + gi
                        mv_t = mvp.tile([P, KNN, D + 1], mdt, tag="mv")
                        nc.sync.dma_start(mv_t, mv[p, g])
                        stT_v = stT[:, gi, :].rearrange("p (ql gf) -> p gf ql",
                                                        gf=KNN)
                        pm_v = pm_ps[:, gi, :].rearrange("p (ql gf) -> p gf ql",
                                                         gf=KNN)
                        for g4 in range(KNN):
                            nc.tensor.matmul(pm_v[:, g4, :], mv_t[:, g4, :],
                                             stT_v[:, g4, :],
                                             start=True, stop=True)
                    pm_sb = pms.tile([D + 1, 4, P], f32, tag="pm")
                    nc.scalar.copy(out=pm_sb, in_=pm_ps)
                    oh_ps = ppt.tile([D, 4, P], f32, tag="tps")
                    for gi in range(4):
                        g = gc + gi
                        qc, gq = g // 4, g % 4
                        nc.tensor.matmul(psum_o[qc][:, gq, :], pm_sb[:, gi, :],
                                         eye_sb[0:D + 1, 0:D + 1],
                                         is_transpose=True, start=_SKIP_LOCAL,
                                         stop=True)
                        rcp = small.tile([P, 1], f32, tag="rcp")
                        nc.vector.reciprocal(rcp, psum_o[qc][:, gq, D:D + 1])
                        oh = small.tile([P, D], f32, tag="oh")
                        nc.vector.tensor_scalar_mul(oh, psum_o[qc][:, gq, 0:D],
                                                    rcp)
                        nc.tensor.transpose(oh_ps[:, gi, :], oh, eye_sb)
                    nc.scalar.copy(out=hoT[p * D:(p + 1) * D, gc:gc + 4, :],
                                   in_=oh_ps)

            _mark(nc, "outproj")
            # ---- output projection (partial: this core's two head rows) --
            for g in range(NB):
                pf = pp512.tile([P, DIM], f32, tag="st")
                nc.tensor.matmul(pf, hoT[:, g, :], wout_sb,
                                 start=True, stop=True)
                of_s = outp.tile([P, DIM], f32, tag="ofs")
                nc.scalar.copy(out=of_s, in_=pf)
                nc.sync.dma_start(out[g * P:(g + 1) * P, :], of_s)

    _mark(nc, "tile_finish")
    nc.compile()
    _mark(nc, None)
    return nc


def _to_bf16(a):
    import ml_dtypes
    return np.ascontiguousarray(a.astype(ml_dtypes.bfloat16))


def _prep_mv(mv_slice):
    """[2,2048,32,64] -> [2,16,128,32,65] bf16: partition (ql j) stacks the 4
    stride-32 queries of each group; col 64 = 1.0 (softmax-denominator row)."""
    import ml_dtypes
    dt = ml_dtypes.bfloat16 if MEMBF16 else np.float32
    r = mv_slice.reshape(2, NB, 4, KNN, KNN, D).transpose(0, 1, 2, 4, 3, 5)
    out = np.empty((2, NB, P, KNN, D + 1), dtype=dt)
    out[..., :D] = r.reshape(2, NB, P, KNN, D).astype(dt)
    out[..., D] = 1.0
    return out


def _prepare_in_maps(x, w_q, w_kv, w_out, scale_param, mem_k, mem_v, mem_mask,
                     use_mbias):
    f = np.float32
    scales8 = np.exp(scale_param.reshape(HEADS).astype(f))
    in_maps = []
    for c in range(NCORES):
        b = c // 4
        h0 = 2 * (c % 4)
        sc = np.empty((P, 4), dtype=f)
        sc[:, 0] = scales8[h0]
        sc[:, 1] = scales8[h0 + 1]
        sc[:, 2] = -scales8[h0]
        sc[:, 3] = -scales8[h0 + 1]
        m = {
            "xb": np.ascontiguousarray(x[b], dtype=f),
            "wq2": np.ascontiguousarray(w_q[:, h0 * D:(h0 + 2) * D], dtype=f),
            "wkv": np.ascontiguousarray(w_kv, dtype=f),
            "wout2": np.ascontiguousarray(w_out[h0 * D:(h0 + 2) * D, :], dtype=f),
            "scales": sc,
            "mk": np.ascontiguousarray(
                mem_k[b, h0:h0 + 2].reshape(2, NB, P, KNN, D), dtype=f),
            "mv": _prep_mv(mem_v[b, h0:h0 + 2]),
        }
        if use_mbias:
            mb = np.where(mem_mask[b, h0:h0 + 2], f(0), f(-1e30)).astype(f)
            m["mbias"] = np.ascontiguousarray(mb.reshape(2, NB, P, KNN))
        in_maps.append(m)
    return in_maps


def _run(x, w_q, w_kv, w_out, scale_param, mem_k, mem_v, mem_mask, trace=False):
    from concourse.bass_utils import run_bass_kernel_spmd

    use_mbias = not bool(np.all(mem_mask))
    nc = _build(use_mbias)
    in_maps = _prepare_in_maps(x, w_q, w_kv, w_out, scale_param,
                               mem_k, mem_v, mem_mask, use_mbias)
    res = run_bass_kernel_spmd(nc, in_maps, core_ids=list(range(NCORES)),
                               trace=trace)
    out = np.zeros((B, N, DIM), dtype=np.float32)
    for c in range(NCORES):
        out[c // 4] += res.results[c]["out"]
    return out, res


def kernel(x, w_q, w_kv, w_out, scale_param, mem_k, mem_v, mem_mask):
    trace = bool(int(os.environ.get("BASS_KERNEL_TRACE", "0")))
    out, _ = _run(x, w_q, w_kv, w_out, scale_param, mem_k, mem_v, mem_mask,
                  trace=trace)
    return out



# revision 38
# speedup vs baseline: 1.7381x; 1.7381x over previous
"""Trainium2 Bass kernel for nn_MemorizingTransformer (retrieval_knn).

Memorizing-transformer attention block: cosine-sim causal local attention with
per-query retrieved KNN memories, joint softmax over [memory | local], and
input/output projections.

Sharding: (b, h) across 8 cores - core c handles batch b=c//4 and heads
h0=2*(c%4), h0+1. Every core runs an identical NEFF (pure SPMD); only input
slices differ. The output projection is computed per-core on the core's two
head rows of w_out, giving partial bf16 sums the host reduces in f32.

Design (vs the f32 baseline):
  * mem_k / mem_v / x / weights / output shipped as bf16 (halves HBM traffic;
    bf16 rather than fp16 because exp(scale*(s-1)) reaches e^-40, far below
    fp16's subnormal floor - a row of all-small scores would flush its whole
    softmax denominator to zero).
  * x pre-transposed on the host - no device transpose phase.
  * q/k/v projections fused into one [512, 256] bf16 matmul chain per block.
  * attention math in bf16 on PE (1 cyc/row) and DVE (2x mode); f32 PSUM.
  * fully qc-pipelined: each 4-block column does proj -> norms -> qT/kT ->
    (per head: mem scores, local stripe, mem values) -> out projection, so
    DVE mem-score work starts ~10us in and out-DMAs spread over the whole
    timeline.
  * ACT emission order puts the local-exp stripe before the (DVE-gated)
    mem-score exp so ACT overlaps DVE instead of serializing behind it.
  * engine balance: DVE = mem scores + sumsq + small scalings; ACT = exps +
    PSUM->SBUF copies; Pool = tril mask + block-diag staging; PE = matmuls.

Softmax needs no max-subtraction: scores are cosine sims in [-1,1] times
scale=exp(scale_param), so exp(scale*(s-1)) is bounded in (0, 1].
"""

import os
import numpy as np

HEADS = 8
D = 64
KNN = 32
B = 2
N = 2048
DIM = 512
P = 128
NB = N // P          # 16 query/key blocks
NCO = DIM // P       # 4 contraction chunks of the model dim
NCORES = 8
PHASE_MARKS = []
_MSTATE = {}


def _mark(nc, name):
    cur = nc.next_id()
    if _MSTATE.get("name") is not None:
        PHASE_MARKS.append((_MSTATE["name"], _MSTATE["id"], cur))
    _MSTATE["name"] = name
    _MSTATE["id"] = cur


def _build(use_mbias: bool):
    import concourse.bass as bass
    import concourse.mybir as mybir
    import concourse.tile as tile
    from concourse import bacc

    f32 = mybir.dt.float32
    f16 = mybir.dt.bfloat16  # 2-byte float: bf16 (fp16 underflows exp(-40))
    AX = mybir.AxisListType
    ACTF = mybir.ActivationFunctionType
    ALU = mybir.AluOpType

    nc = bacc.Bacc(None, target_bir_lowering=False, name="memxformer")
    PHASE_MARKS.clear()
    _MSTATE.clear()

    # ---- I/O ------------------------------------------------------------
    xT_d = nc.dram_tensor("xT", (DIM, N), f16, kind="ExternalInput")
    wqkv_d = nc.dram_tensor("wqkv", (DIM, 4 * D), f16, kind="ExternalInput")
    wout_d = nc.dram_tensor("wout2", (2 * D, DIM), f16, kind="ExternalInput")
    # scales[:, 0:2] = exp(scale_param[h0 + p]); scales[:, 2:4] = -that
    scales = nc.dram_tensor("scales", (P, 4), f32, kind="ExternalInput")
    mk = nc.dram_tensor("mk", (2, NB, P, KNN, D), f16, kind="ExternalInput")
    mv = nc.dram_tensor("mv", (2, NB, P, KNN, D + 1), f16, kind="ExternalInput")
    if use_mbias:
        mbias = nc.dram_tensor("mbias", (2, NB, P, KNN), f32, kind="ExternalInput")
    out = nc.dram_tensor("out", (N, DIM), f16, kind="ExternalOutput")

    # constants baked into the NEFF
    eye_np = np.eye(P, dtype=np.float32)
    tril_np = np.triu(np.ones((P, P), dtype=np.float32))  # keep j <= q
    import ml_dtypes
    eye_d = nc.inline_tensor(eye_np, name="eye_c")
    eye16_d = nc.inline_tensor(eye_np.astype(ml_dtypes.bfloat16), name="eye16_c")
    tril16_d = nc.inline_tensor(tril_np.astype(ml_dtypes.bfloat16), name="tril16_c")

    with tile.TileContext(nc) as tc:
        with (
            tc.tile_pool(name="singles", bufs=1) as singles,
            tc.tile_pool(name="mkp", bufs=3) as mkp,
            tc.tile_pool(name="prodp", bufs=1) as prodp,
            tc.tile_pool(name="h1p", bufs=1) as h1p,
            tc.tile_pool(name="mvp", bufs=8) as mvp,
            tc.tile_pool(name="small", bufs=6) as small,
            tc.tile_pool(name="pt", bufs=3) as ptp,
            tc.tile_pool(name="stts", bufs=2) as stts,
            tc.tile_pool(name="pms", bufs=2) as pms,
            tc.tile_pool(name="outp", bufs=9) as outp,
            tc.tile_pool(name="pmem", bufs=3) as pmemp,
            tc.tile_pool(name="ppt", bufs=2, space="PSUM") as ppt,
            tc.tile_pool(name="pp512", bufs=2, space="PSUM") as pp512,
            tc.tile_pool(name="ppo", bufs=3, space="PSUM") as ppo,
            tc.tile_pool(name="pprj", bufs=1, space="PSUM") as pprj,
        ):
            # ---- constants / weights (issue order = need order) ---------
            wqkv_sb = singles.tile([P, NCO, 4 * D], f16, tag="wqkv")
            nc.sync.dma_start(wqkv_sb,
                              wqkv_d[:, :].rearrange("(co p) c -> p co c", p=P))
            eye16_sb = singles.tile([P, P], f16, tag="eye16")
            nc.sync.dma_start(eye16_sb, eye16_d[:, :])
            sc_sb = singles.tile([P, 4], f32, tag="scales")
            nc.sync.dma_start(sc_sb, scales[:, :])
            tril16_sb = singles.tile([P, P], f16, tag="tril16")
            nc.sync.dma_start(tril16_sb, tril16_d[:, :])
            eye_sb = singles.tile([P, P], f32, tag="eye")
            nc.sync.dma_start(eye_sb, eye_d[:, :])
            wout16 = singles.tile([P, DIM], f16, tag="wout16")
            nc.sync.dma_start(wout16, wout_d[:, :])

            _mark(nc, "setup")
            xT = singles.tile([P, NCO, N], f16, tag="xT")
            # kv_all cols: 0:64 = k (natural), 64:128 = v, 128 = ones
            kv_all = singles.tile([P, NB, 2 * D + 1], f16, tag="kv_all")
            nc.gpsimd.memset(kv_all[:, :, 2 * D:2 * D + 1], 1.0)
            # q_all g-major so both heads' q for block g land in one copy
            q_all = singles.tile([P, NB, 2, D], f16, tag="q_all")
            # ss_all[:, g] = (k, q0, q1) sums of squares for block g
            ss_all = singles.tile([P, NB, 3], f32, tag="ss")
            rn = singles.tile([P, NB, 3], f32, tag="rn")
            junk = singles.tile([P, D], f16, tag="junk")
            q_s = singles.tile([P, 2 * NB, D], f16, tag="q_s")
            qT = singles.tile([D, 2 * NB, P], f16, tag="qT")
            kT = singles.tile([D, NB, P], f16, tag="kT")
            hoT = singles.tile([P, NB, P], f16, tag="hoT")
            # staging for block-diagonal P_mem (one buffer per head; the
            # off-diagonal zeros are written once and never touched again)
            st2 = singles.tile([P, 2, 4, P], f16, tag="st2")
            nc.gpsimd.memset(st2, 0.0)

            _mark(nc, "main")

            def emit_combine(po, p, gc0):
                """Normalize psum_o -> hoT rows for head p, column gc0."""
                oh_ps = ppt.tile([D, 4, P], f16, tag="tps")
                for gi in range(4):
                    rcp = small.tile([P, 1], f32, tag="rcp")
                    nc.vector.reciprocal(rcp, po[:, gi, D:D + 1])
                    oh = small.tile([P, D], f16, tag="oh")
                    nc.vector.tensor_scalar_mul(oh, po[:, gi, 0:D], rcp)
                    nc.tensor.transpose(oh_ps[:, gi, :], oh, eye16_sb)
                nc.scalar.copy(out=hoT[p * D:(p + 1) * D, gc0:gc0 + 4, :],
                               in_=oh_ps)

            pending_out = []

            def emit_outproj(gc0):
                for gi in range(4):
                    g = gc0 + gi
                    pf = pp512.tile([P, DIM], f32, tag="big")
                    nc.tensor.matmul(pf, hoT[:, g, :], wout16,
                                     start=True, stop=True)
                    of_s = outp.tile([P, DIM], f16, tag="ofs")
                    nc.scalar.copy(out=of_s, in_=pf)
                    pending_out.append((g, of_s))

            def flush_out(keep=0):
                # SP-queue out-writes, issued ~a column after their of_s was
                # produced so they never head-of-line block the mk/mv stream
                while len(pending_out) > keep:
                    g, of_s = pending_out.pop(0)
                    nc.sync.dma_start(out[g * P:(g + 1) * P, :], of_s)

            def load_xt(qc):
                nsl = slice(qc * 4 * P, (qc + 1) * 4 * P)
                nc.sync.dma_start(
                    xT[:, :, nsl],
                    xT_d[:, nsl].rearrange("(co p) n -> p co n", p=P))

            load_xt(0)
            prev_p1 = None  # (psum_o, gc) of last column's head-1, pending
            for qc in range(4):
                gc = 4 * qc
                for g in range(gc, gc + 4):
                    gsl = slice(g * P, (g + 1) * P)
                    qkv_ps = pprj.tile([P, 4 * D], f32, tag="qkv")
                    for co in range(NCO):
                        nc.tensor.matmul(qkv_ps, xT[:, co, gsl],
                                         wqkv_sb[:, co, :],
                                         start=(co == 0), stop=(co == NCO - 1))
                    nc.scalar.copy(out=kv_all[:, g, 0:2 * D],
                                   in_=qkv_ps[:, 0:2 * D])
                    nc.scalar.copy(
                        out=q_all[:, g, :, :].rearrange("p t d -> p (t d)"),
                        in_=qkv_ps[:, 2 * D:4 * D])
                    # sumsq (k, q0, q1) from the bf16 SBUF copies on DVE
                    # (square into junk, then reduce; TensorTensorReduce
                    # faults at runtime on this HW/runtime combo)
                    nc.vector.tensor_mul(junk, kv_all[:, g, 0:D],
                                         kv_all[:, g, 0:D])
                    nc.vector.reduce_sum(ss_all[:, g, 0:1], junk, axis=AX.X)
                    for p in range(2):
                        nc.vector.tensor_mul(junk, q_all[:, g, p, :],
                                             q_all[:, g, p, :])
                        nc.vector.reduce_sum(ss_all[:, g, 1 + p:2 + p], junk,
                                             axis=AX.X)

                # ---- norms + scaled q/k + transposes for this column ----
                nrm = small.tile([P, 4, 3], f32, tag="nrm")
                nc.scalar.sqrt(nrm, ss_all[:, gc:gc + 4, :])
                nc.vector.reciprocal(rn[:, gc:gc + 4, :], nrm)

                for ph in range(2):
                    qt_ps = ppt.tile([D, 4, P], f16, tag="tps")
                    for i4 in range(4):
                        g = gc + i4
                        idx = ph * NB + g
                        nc.vector.tensor_scalar_mul(
                            q_s[:, idx, :], q_all[:, g, ph, :],
                            rn[:, g, 1 + ph:2 + ph])
                        nc.tensor.transpose(qt_ps[:, i4, :], q_s[:, idx, :],
                                            eye16_sb)
                    nc.scalar.copy(out=qT[:, ph * NB + gc:ph * NB + gc + 4, :],
                                   in_=qt_ps)
                kt_ps = ppt.tile([D, 4, P], f16, tag="tps")
                for j4 in range(4):
                    jt = gc + j4
                    ktmp = small.tile([P, D], f16, tag="ktmp")
                    nc.vector.tensor_scalar_mul(ktmp, kv_all[:, jt, 0:D],
                                                rn[:, jt, 0:1])
                    nc.tensor.transpose(kt_ps[:, j4, :], ktmp, eye16_sb)
                nc.scalar.copy(out=kT[:, gc:gc + 4, :], in_=kt_ps)

                for p in range(2):
                    sc_ap = sc_sb[:, p:p + 1]
                    nb_ap = sc_sb[:, 2 + p:3 + p]

                    # --- mem scores for this column (DVE) -----------------
                    mk_t = mkp.tile([P, 4, KNN, D], f16, tag="mk")
                    if qc == 0 and p == 0:
                        # split the very first mk transfer so scoring starts
                        # as soon as the first half lands
                        for hh in range(2):
                            nc.sync.dma_start(
                                mk_t[:, 2 * hh:2 * hh + 2],
                                mk[p, 2 * hh:2 * hh + 2]
                                .rearrange("g p k d -> p g k d"))
                    else:
                        nc.sync.dma_start(
                            mk_t, mk[p, gc:gc + 4].rearrange("g p k d -> p g k d"))
                    # prefetch this section's mem-value tiles right behind
                    mv_ts = []
                    for gi in range(4):
                        mv_t = mvp.tile([P, KNN, D + 1], f16, tag="mv")
                        nc.sync.dma_start(mv_t, mv[p, gc + gi])
                        mv_ts.append(mv_t)
                    if qc == 0 and p == 0:
                        # remaining x chunks ride right behind the first
                        # mem tiles; all later columns' projections decouple
                        # from the mem-stream queue.
                        for xc in range(1, 4):
                            load_xt(xc)
                    flush_out()
                    # mul then one fp16 pairwise-add level (2x DVE mode)
                    # before the f32 segmented reduce (which has no 2x).
                    prod = prodp.tile([P, 4, KNN, D], f16, tag="prod")
                    h1 = h1p.tile([P, 4, KNN, D // 2], f16, tag="h1")
                    s_mem = small.tile([P, 4, KNN], f32, tag="smem")
                    split = (qc == 3 and p == 1)  # last section: per-block
                    if qc == 0 and p == 0:
                        halves = (2, 2)
                    elif split:
                        halves = (1, 1, 1, 1)
                    else:
                        halves = (4,)
                    g0 = 0
                    for nh in halves:
                        hs = slice(g0, g0 + nh)
                        nc.vector.tensor_mul(
                            prod[:, hs], mk_t[:, hs],
                            q_s[:, p * NB + gc + g0:p * NB + gc + g0 + nh,
                                None, :].to_broadcast((P, nh, KNN, D)))
                        nc.vector.tensor_add(h1[:, hs], prod[:, hs, :, 0:D // 2],
                                             prod[:, hs, :, D // 2:D])
                        nc.vector.reduce_sum(s_mem[:, hs], h1[:, hs], axis=AX.X)
                        g0 += nh
                    if use_mbias:
                        mb_t = small.tile([P, 4, KNN], f32, tag="mbias")
                        nc.sync.dma_start(
                            mb_t,
                            mbias[p, gc:gc + 4].rearrange("g p k -> p g k"))
                        nc.vector.tensor_add(s_mem, s_mem, mb_t)

                    # --- local causal attention stripe (PE + ACT) ---------
                    psum_o = ppo.tile([P, 4, D + 1], f32, tag="po")
                    for jt in range(4 * qc + 4):
                        g_lo = max(jt, gc)
                        ng = gc + 4 - g_lo
                        i_lo = p * NB + g_lo
                        st_ps = pp512.tile([P, 512], f32, tag="big",
                                           name="st_ps")
                        nc.tensor.matmul(
                            st_ps[:, :ng * P], kT[:, jt, :],
                            qT[:, i_lo:i_lo + ng, :],
                            start=True, stop=True)
                        p_t = ptp.tile([P, 4, P], f16, tag="pt", name="p_t")
                        nc.scalar.activation(
                            out=p_t[:, :ng, :],
                            in_=st_ps[:, :ng * P].rearrange("p (g q) -> p g q",
                                                            q=P),
                            func=ACTF.Exp, bias=nb_ap, scale=sc_ap)
                        if g_lo <= jt:
                            di = jt - g_lo
                            nc.gpsimd.tensor_mul(p_t[:, di, :], p_t[:, di, :],
                                                 tril16_sb)
                        for gi in range(ng):
                            g = g_lo + gi
                            nc.tensor.matmul(
                                psum_o[:, g - gc, :], p_t[:, gi, :],
                                kv_all[:, jt, D:2 * D + 1],
                                start=(jt == 0 and gi == 0), stop=False)

                    # --- mem scores exp + mem values (block-diag PE trick) -
                    # pm stored gf-major so each 4-query matmul writes a
                    # CONTIGUOUS psum run; the pm_sb copy permutes back to
                    # ql-major so the accumulate's stationary is one
                    # contiguous free dim.
                    stage4 = st2[:, p, :, :]
                    p_mem = pmemp.tile([P, 4, KNN], f16, tag="pmem")

                    def memv_chain(gis):
                        ngi = len(gis)
                        nc.scalar.activation(
                            out=p_mem[:, gis[0]:gis[0] + ngi, :]
                            .rearrange("p g k -> p (g k)"),
                            in_=s_mem[:, gis[0]:gis[0] + ngi, :]
                            .rearrange("p g k -> p (g k)"),
                            func=ACTF.Exp, bias=nb_ap, scale=sc_ap)
                        for gi in gis:
                            for k4 in range(4):
                                nc.gpsimd.tensor_copy(
                                    out=stage4[32 * k4:32 * (k4 + 1), gi,
                                               32 * k4:32 * (k4 + 1)],
                                    in_=p_mem[32 * k4:32 * (k4 + 1), gi, :])
                        stt_ps = ppt.tile([P, ngi, P], f16, tag="tps")
                        for i, gi in enumerate(gis):
                            nc.tensor.transpose(stt_ps[:, i, :],
                                                stage4[:, gi, :], eye16_sb)
                        stT = stts.tile([P, ngi, P], f16, tag="stT")
                        nc.scalar.copy(out=stT, in_=stt_ps)
                        pm_ps = pp512.tile([D + 1, ngi, KNN, 4], f32, tag="big")
                        for i, gi in enumerate(gis):
                            mv_t = mv_ts[gi]
                            stT_v = stT[:, i, :].rearrange(
                                "p (ql gf) -> p gf ql", gf=KNN)
                            for g4 in range(KNN):
                                nc.tensor.matmul(pm_ps[:, i, g4, :],
                                                 mv_t[:, g4, :], stT_v[:, g4, :],
                                                 start=True, stop=True)
                        pm_sb = pms.tile([D + 1, ngi, 4, KNN], f32, tag="pm")
                        nc.scalar.copy(
                            out=pm_sb.rearrange("p a ql gf -> p a gf ql"),
                            in_=pm_ps)
                        for i, gi in enumerate(gis):
                            nc.tensor.matmul(psum_o[:, gi, :],
                                             pm_sb[:, i, :, :],
                                             eye_sb[0:D + 1, 0:D + 1],
                                             is_transpose=True, start=False,
                                             stop=(gi == 3))

                    if split:
                        for gi in range(4):
                            memv_chain([gi])
                    else:
                        memv_chain([0, 1, 2, 3])

                    # Deferred combines keep DVE's in-order stream out of the
                    # cross-engine mem-value chain: after head-0's mem values,
                    # finish the PREVIOUS column's head-1 (+its out rows);
                    # after head-1's, finish this column's head-0. The last
                    # column finishes head-0 early and head-1 inline so the
                    # tail is one short per-block chain.
                    if p == 0:
                        if prev_p1 is not None:
                            emit_combine(prev_p1[0], 1, prev_p1[1])
                            emit_outproj(prev_p1[1])
                        psum_p0 = psum_o
                        if qc == 3:
                            emit_combine(psum_o, 0, gc)
                    elif qc < 3:
                        emit_combine(psum_p0, 0, gc)
                        prev_p1 = (psum_o, gc)
                    else:
                        emit_combine(psum_o, 1, gc)
                        emit_outproj(gc)
            flush_out()

    _mark(nc, "tile_finish")
    nc.compile()
    _mark(nc, None)
    return nc


def _prep_mv(mv_slice):
    """[2,2048,32,64] -> [2,16,128,32,65] fp16: partition (jj K) stacks the 4
    stride-32 queries of each group; col 64 = 1.0 (softmax-denominator row)."""
    r = mv_slice.reshape(2, NB, 4, KNN, KNN, D).transpose(0, 1, 2, 4, 3, 5)
    import ml_dtypes
    o = np.empty((2, NB, P, KNN, D + 1), dtype=ml_dtypes.bfloat16)
    o[..., :D] = r.reshape(2, NB, P, KNN, D).astype(ml_dtypes.bfloat16)
    o[..., D] = 1.0
    return o


def _prepare_in_maps(x, w_q, w_kv, w_out, scale_param, mem_k, mem_v, mem_mask,
                     use_mbias):
    f = np.float32
    import ml_dtypes
    f16 = ml_dtypes.bfloat16
    scales8 = np.exp(scale_param.reshape(HEADS).astype(f))
    in_maps = []
    for c in range(NCORES):
        b = c // 4
        h0 = 2 * (c % 4)
        sc = np.empty((P, 4), dtype=f)
        sc[:, 0] = scales8[h0]
        sc[:, 1] = scales8[h0 + 1]
        sc[:, 2] = -scales8[h0]
        sc[:, 3] = -scales8[h0 + 1]
        m = {
            "xT": np.ascontiguousarray(x[b].T.astype(f16)),
            "wqkv": np.ascontiguousarray(
                np.concatenate([w_kv, w_q[:, h0 * D:(h0 + 2) * D]],
                               axis=1).astype(f16)),
            "wout2": np.ascontiguousarray(
                w_out[h0 * D:(h0 + 2) * D, :].astype(f16)),
            "scales": sc,
            "mk": np.ascontiguousarray(
                mem_k[b, h0:h0 + 2].reshape(2, NB, P, KNN, D).astype(f16)),
            "mv": _prep_mv(mem_v[b, h0:h0 + 2]),
        }
        if use_mbias:
            mb = np.where(mem_mask[b, h0:h0 + 2], f(0), f(-1e30)).astype(f)
            m["mbias"] = np.ascontiguousarray(mb.reshape(2, NB, P, KNN))
        in_maps.append(m)
    return in_maps


def _run(x, w_q, w_kv, w_out, scale_param, mem_k, mem_v, mem_mask, trace=False):
    from concourse.bass_utils import run_bass_kernel_spmd

    use_mbias = not bool(np.all(mem_mask))
    nc = _build(use_mbias)
    in_maps = _prepare_in_maps(x, w_q, w_kv, w_out, scale_param,
                               mem_k, mem_v, mem_mask, use_mbias)
    res = run_bass_kernel_spmd(nc, in_maps, core_ids=list(range(NCORES)),
                               trace=trace)
    out = np.zeros((B, N, DIM), dtype=np.float32)
    for c in range(NCORES):
        out[c // 4] += res.results[c]["out"].astype(np.float32)
    return out, res


def kernel(x, w_q, w_kv, w_out, scale_param, mem_k, mem_v, mem_mask):
    trace = bool(int(os.environ.get("BASS_KERNEL_TRACE", "0")))
    out, _ = _run(x, w_q, w_kv, w_out, scale_param, mem_k, mem_v, mem_mask,
                  trace=trace)
    return out
